# revision 1
# baseline (speedup 1.0000x reference)
"""BlockwiseSelector Trainium2 kernel.

Computes, for q (B,H,N,D) and compressed_k (B,H,M,D):
  scores = softmax(q @ ck^T / sqrt(D), axis=-1)        -> (B,H,N,M) fp32
  importance = scores.sum(axis=1)                      -> (B,N,M)
  top_indices = top_k(importance, 16).indices          -> (B,N,16) int32
Returns (top_indices, scores).

Sharding: data-parallel over (batch, N-half) -> 8 cores, each core owns
(b = core//2, rows n in [half*2048, half*2048+2048)) for all 16 heads.
k and v inputs are unused by the reference computation.
"""

import numpy as np

B, H, N, D, M, TOPN = 4, 16, 4096, 64, 256, 16
NCORES = 8
NL = N // 2  # rows per core
CH = NL // 128  # chunks of 128 rows per core
SCALE = float(D) ** -0.5

_NC_CACHE = {}


def _build():
    import concourse.tile as tile
    from concourse import bacc, mybir

    nc = bacc.Bacc(name="blockwise_selector")
    qc = nc.declare_dram_parameter(
        "qc", [CH, D, H * 128], mybir.dt.float32, isOutput=False
    )
    ck = nc.declare_dram_parameter("ck", [D, H * M], mybir.dt.float32, isOutput=False)
    probs = nc.declare_dram_parameter(
        "probs", [H, NL, M], mybir.dt.float32, isOutput=True
    )
    idx = nc.declare_dram_parameter("idx", [NL, TOPN], mybir.dt.uint32, isOutput=True)

    with tile.TileContext(nc) as tc:
        with (
            tc.tile_pool(name="ckpool", bufs=1) as ckpool,
            tc.tile_pool(name="qpool", bufs=3) as qpool,
            tc.tile_pool(name="epool", bufs=3) as epool,
            tc.tile_pool(name="tpool", bufs=2) as tpool,
            tc.tile_pool(name="spool", bufs=3) as spool,
            tc.tile_pool(name="pspool", bufs=6, space="PSUM") as pspool,
        ):
            ck_sb = ckpool.tile([D, H * M], mybir.dt.float32)
            nc.sync.dma_start(ck_sb[:], ck[:])
            for c in range(CH):
                qc_sb = qpool.tile([D, H * 128], mybir.dt.float32)
                nc.sync.dma_start(qc_sb[:], qc[c])
                eb = epool.tile([128, H * M], mybir.dt.float32)
                sums = spool.tile([128, H], mybir.dt.float32)
                for h in range(H):
                    ps = pspool.tile([128, M], mybir.dt.float32)
                    nc.tensor.matmul(
                        ps[:],
                        qc_sb[:, h * 128 : (h + 1) * 128],
                        ck_sb[:, h * M : (h + 1) * M],
                        start=True,
                        stop=True,
                    )
                    nc.scalar.activation(
                        eb[:, h * M : (h + 1) * M],
                        ps[:],
                        mybir.ActivationFunctionType.Exp,
                        scale=SCALE,
                        accum_out=sums[:, h : h + 1],
                    )
                rec = spool.tile([128, H], mybir.dt.float32)
                nc.vector.reciprocal(rec[:], sums[:])
                for h in range(H):
                    nc.vector.tensor_scalar_mul(
                        eb[:, h * M : (h + 1) * M],
                        eb[:, h * M : (h + 1) * M],
                        rec[:, h : h + 1],
                    )
                nc.sync.dma_start(
                    probs[:, c * 128 : (c + 1) * 128, :].rearrange("h n m -> n h m"),
                    eb[:].rearrange("n (h m) -> n h m", h=H),
                )
                # head-sum reduction tree (fp32 exact)
                s1 = tpool.tile([128, H * M // 2], mybir.dt.float32)
                nc.vector.tensor_add(s1[:], eb[:, : H * M // 2], eb[:, H * M // 2 :])
                nc.vector.tensor_add(s1[:, :1024], s1[:, :1024], s1[:, 1024:2048])
                nc.vector.tensor_add(s1[:, :512], s1[:, :512], s1[:, 512:1024])
                imp = spool.tile([128, M], mybir.dt.float32)
                nc.vector.tensor_add(imp[:], s1[:, :256], s1[:, 256:512])
                # top-16 of 256 per row: two rounds of max8 + index + replace
                mx1 = spool.tile([128, 8], mybir.dt.float32)
                nc.vector.max(mx1[:], imp[:])
                idxs = spool.tile([128, TOPN], mybir.dt.uint32)
                nc.vector.max_index(idxs[:, 0:8], mx1[:], imp[:])
                imp2 = spool.tile([128, M], mybir.dt.float32)
                nc.vector.match_replace(imp2[:], mx1[:], imp[:], imm_value=-1.0)
                mx2 = spool.tile([128, 8], mybir.dt.float32)
                nc.vector.max(mx2[:], imp2[:])
                nc.vector.max_index(idxs[:, 8:16], mx2[:], imp2[:])
                nc.sync.dma_start(idx[c * 128 : (c + 1) * 128, :], idxs[:])
    nc.finalize()
    return nc


def _get_nc():
    if "nc" not in _NC_CACHE:
        _NC_CACHE["nc"] = _build()
    return _NC_CACHE["nc"]


def _make_in_maps(q, compressed_k):
    in_maps = []
    for core in range(NCORES):
        b, half = core // 2, core % 2
        qs = q[b, :, half * NL : (half + 1) * NL, :]  # (H, NL, D)
        qc = (
            qs.reshape(H, CH, 128, D).transpose(1, 3, 0, 2).reshape(CH, D, H * 128)
        )
        ckT = compressed_k[b].transpose(2, 0, 1).reshape(D, H * M)
        in_maps.append(
            {
                "qc": np.ascontiguousarray(qc),
                "ck": np.ascontiguousarray(ckT),
            }
        )
    return in_maps


def _run(q, compressed_k, trace=False):
    from concourse.bass_utils import run_bass_kernel_spmd

    nc = _get_nc()
    in_maps = _make_in_maps(q, compressed_k)
    res = run_bass_kernel_spmd(
        nc, in_maps, core_ids=list(range(NCORES)), trace=trace
    )
    cs = np.empty((B, H, N, M), np.float32)
    ti = np.empty((B, N, TOPN), np.int32)
    for core in range(NCORES):
        b, half = core // 2, core % 2
        cs[b, :, half * NL : (half + 1) * NL, :] = res.results[core]["probs"]
        ti[b, half * NL : (half + 1) * NL, :] = res.results[core]["idx"].astype(
            np.int32
        )
    return (ti, cs), res


def kernel(q, compressed_k, k=None, v=None):
    q = np.ascontiguousarray(np.asarray(q, dtype=np.float32))
    compressed_k = np.ascontiguousarray(np.asarray(compressed_k, dtype=np.float32))
    (ti, cs), _ = _run(q, compressed_k, trace=False)
    return ti, cs


# revision 3
# speedup vs baseline: 1.1979x; 1.1979x over previous
"""BlockwiseSelector Trainium2 kernel (v2).

Computes, for q (B,H,N,D) and compressed_k (B,H,M,D):
  scores = softmax(q @ ck^T / sqrt(D), axis=-1)        -> (B,H,N,M) fp32
  importance = scores.sum(axis=1)                      -> (B,N,M)
  top_indices = top_k(importance, 16).indices          -> (B,N,16) int32
Returns (top_indices, scores).

Sharding: data-parallel over (batch, N-half) -> 8 cores, each core owns
(b = core//2, rows n in [half*2048, half*2048+2048)) for all 16 heads.
k and v inputs are unused by the reference computation.

Numerics: q (pre-scaled by 1/sqrt(D)) and ck are split on host into bf16
hi+lo; QK^T runs as 3 bf16 matmuls (hi*hi + hi*lo + lo*hi) accumulated in
fp32 PSUM, carrying ~2^-17 relative error on scores (fp32-class softmax
quality). Softmax skips max-subtraction: |scores| <= ~9 so exp() is safe.

Engine split per 128-row chunk:
  PE:  48 bf16 matmuls (3 per head)
  ACT: 16x per-head Exp from PSUM with accum_out row-sums
  DVE: reciprocal, 16x normalize (tensor_scalar by 1/S), head-sum tree,
       top-16 via max8/max_index/match_replace
  DMA: chunk-contiguous output (16KB descriptors), host re-layouts.
"""

import numpy as np

B, H, N, D, M, TOPN = 4, 16, 4096, 64, 256, 16
NCORES = 8
NL = N // 2  # rows per core
CH = NL // 128  # chunks of 128 rows per core
SCALE = float(D) ** -0.5

_NC_CACHE = {}


def _build():
    import concourse.tile as tile
    from concourse import bacc, mybir

    nc = bacc.Bacc(name="blockwise_selector")
    f32 = mybir.dt.float32
    bf16 = mybir.dt.bfloat16
    qhi = nc.declare_dram_parameter("qhi", [CH, D, H * 128], bf16, isOutput=False)
    qlo = nc.declare_dram_parameter("qlo", [CH, D, H * 128], bf16, isOutput=False)
    ckhi = nc.declare_dram_parameter("ckhi", [D, H * M], bf16, isOutput=False)
    cklo = nc.declare_dram_parameter("cklo", [D, H * M], bf16, isOutput=False)
    probs = nc.declare_dram_parameter("probs", [CH, 128, H * M], f32, isOutput=True)
    idx = nc.declare_dram_parameter("idx", [NL, TOPN], mybir.dt.uint32, isOutput=True)

    with tile.TileContext(nc) as tc:
        with (
            tc.tile_pool(name="ckpool", bufs=1) as ckpool,
            tc.tile_pool(name="qpool", bufs=3) as qpool,
            tc.tile_pool(name="epool", bufs=3) as epool,
            tc.tile_pool(name="tpool", bufs=2) as tpool,
            tc.tile_pool(name="spool", bufs=3) as spool,
            tc.tile_pool(name="pspool", bufs=8, space="PSUM") as pspool,
        ):
            ckh_sb = ckpool.tile([D, H * M], bf16)
            nc.sync.dma_start(ckh_sb[:], ckhi[:])
            ckl_sb = ckpool.tile([D, H * M], bf16)
            nc.sync.dma_start(ckl_sb[:], cklo[:])
            for c in range(CH):
                qh_sb = qpool.tile([D, H * 128], bf16)
                nc.sync.dma_start(qh_sb[:], qhi[c])
                ql_sb = qpool.tile([D, H * 128], bf16)
                nc.sync.dma_start(ql_sb[:], qlo[c])
                eb = epool.tile([128, H * M], f32)
                sums = spool.tile([128, H], f32)
                for h in range(H):
                    qh_ap = qh_sb[:, h * 128 : (h + 1) * 128]
                    ql_ap = ql_sb[:, h * 128 : (h + 1) * 128]
                    ckh_ap = ckh_sb[:, h * M : (h + 1) * M]
                    ckl_ap = ckl_sb[:, h * M : (h + 1) * M]
                    ps = pspool.tile([128, M], f32)
                    nc.tensor.matmul(ps[:], qh_ap, ckh_ap, start=True, stop=False)
                    nc.tensor.matmul(ps[:], qh_ap, ckl_ap, start=False, stop=False)
                    nc.tensor.matmul(ps[:], ql_ap, ckh_ap, start=False, stop=True)
                    nc.scalar.activation(
                        eb[:, h * M : (h + 1) * M],
                        ps[:],
                        mybir.ActivationFunctionType.Exp,
                        accum_out=sums[:, h : h + 1],
                    )
                rec = spool.tile([128, H], f32)
                nc.vector.reciprocal(rec[:], sums[:])
                for h in range(H):
                    src = eb[:, h * M : (h + 1) * M]
                    nc.vector.tensor_scalar_mul(src, src, rec[:, h : h + 1])
                nc.sync.dma_start(probs[c], eb[:])
                # head-sum reduction tree (fp32 exact)
                s1 = tpool.tile([128, H * M // 2], f32)
                nc.vector.tensor_add(s1[:], eb[:, : H * M // 2], eb[:, H * M // 2 :])
                nc.vector.tensor_add(s1[:, :1024], s1[:, :1024], s1[:, 1024:2048])
                nc.vector.tensor_add(s1[:, :512], s1[:, :512], s1[:, 512:1024])
                imp = spool.tile([128, M], f32)
                nc.vector.tensor_add(imp[:], s1[:, :256], s1[:, 256:512])
                # top-16 of 256 per row: two rounds of max8 + index + replace
                mx1 = spool.tile([128, 8], f32)
                nc.vector.max(mx1[:], imp[:])
                idxs = spool.tile([128, TOPN], mybir.dt.uint32)
                nc.vector.max_index(idxs[:, 0:8], mx1[:], imp[:])
                imp2 = spool.tile([128, M], f32)
                nc.vector.match_replace(imp2[:], mx1[:], imp[:], imm_value=-1.0)
                mx2 = spool.tile([128, 8], f32)
                nc.vector.max(mx2[:], imp2[:])
                nc.vector.max_index(idxs[:, 8:16], mx2[:], imp2[:])
                nc.sync.dma_start(idx[c * 128 : (c + 1) * 128, :], idxs[:])
    nc.finalize()
    return nc


def _get_nc():
    if "nc" not in _NC_CACHE:
        _NC_CACHE["nc"] = _build()
    return _NC_CACHE["nc"]


def _bf16_split(x):
    import ml_dtypes

    hi = x.astype(ml_dtypes.bfloat16)
    lo = (x - hi.astype(np.float32)).astype(ml_dtypes.bfloat16)
    return hi, lo


def _make_in_maps(q, compressed_k):
    in_maps = []
    for core in range(NCORES):
        b, half = core // 2, core % 2
        qs = q[b, :, half * NL : (half + 1) * NL, :] * np.float32(SCALE)  # (H,NL,D)
        qc = qs.reshape(H, CH, 128, D).transpose(1, 3, 0, 2).reshape(CH, D, H * 128)
        ckT = compressed_k[b].transpose(2, 0, 1).reshape(D, H * M)
        qh, ql = _bf16_split(np.ascontiguousarray(qc))
        ckh, ckl = _bf16_split(np.ascontiguousarray(ckT))
        in_maps.append({"qhi": qh, "qlo": ql, "ckhi": ckh, "cklo": ckl})
    return in_maps


def _run(q, compressed_k, trace=False):
    from concourse.bass_utils import run_bass_kernel_spmd

    nc = _get_nc()
    in_maps = _make_in_maps(q, compressed_k)
    res = run_bass_kernel_spmd(
        nc, in_maps, core_ids=list(range(NCORES)), trace=trace
    )
    cs = np.empty((B, H, N, M), np.float32)
    ti = np.empty((B, N, TOPN), np.int32)
    for core in range(NCORES):
        b, half = core // 2, core % 2
        pr = res.results[core]["probs"]  # (CH, 128, H*M)
        pr = pr.reshape(CH, 128, H, M).transpose(2, 0, 1, 3).reshape(H, NL, M)
        cs[b, :, half * NL : (half + 1) * NL, :] = pr
        ti[b, half * NL : (half + 1) * NL, :] = res.results[core]["idx"].astype(
            np.int32
        )
    return (ti, cs), res


def kernel(q, compressed_k, k=None, v=None):
    q = np.ascontiguousarray(np.asarray(q, dtype=np.float32))
    compressed_k = np.ascontiguousarray(np.asarray(compressed_k, dtype=np.float32))
    (ti, cs), _ = _run(q, compressed_k, trace=False)
    return ti, cs


# revision 6
# speedup vs baseline: 1.2560x; 1.0485x over previous
"""BlockwiseSelector Trainium2 kernel (v2).

Computes, for q (B,H,N,D) and compressed_k (B,H,M,D):
  scores = softmax(q @ ck^T / sqrt(D), axis=-1)        -> (B,H,N,M) fp32
  importance = scores.sum(axis=1)                      -> (B,N,M)
  top_indices = top_k(importance, 16).indices          -> (B,N,16) int32
Returns (top_indices, scores).

Sharding: data-parallel over (batch, N-half) -> 8 cores, each core owns
(b = core//2, rows n in [half*2048, half*2048+2048)) for all 16 heads.
k and v inputs are unused by the reference computation.

Numerics: q (pre-scaled by 1/sqrt(D)) and ck are split on host into bf16
hi+lo; QK^T runs as 3 bf16 matmuls (hi*hi + hi*lo + lo*hi) accumulated in
fp32 PSUM, carrying ~2^-17 relative error on scores (fp32-class softmax
quality). Softmax skips max-subtraction: |scores| <= ~9 so exp() is safe.

Engine split per 128-row chunk:
  PE:  48 bf16 matmuls (3 per head)
  ACT: 16x per-head Exp from PSUM with accum_out row-sums
  DVE: reciprocal, 16x normalize (tensor_scalar by 1/S), head-sum tree,
       top-16 via max8/max_index/match_replace
  DMA: chunk-contiguous output (16KB descriptors), host re-layouts.
"""

import numpy as np

B, H, N, D, M, TOPN = 4, 16, 4096, 64, 256, 16
NCORES = 8
NL = N // 2  # rows per core
CH = NL // 128  # chunks of 128 rows per core
SCALE = float(D) ** -0.5

_NC_CACHE = {}


def _build():
    import concourse.tile as tile
    from concourse import bacc, mybir

    nc = bacc.Bacc(name="blockwise_selector")
    f32 = mybir.dt.float32
    bf16 = mybir.dt.bfloat16
    # q hi/lo stacked along contraction dim (partitions 0-63 hi, 64-127 lo);
    # ck hi (resp. lo) replicated in both halves -> 2 K=128 matmuls per head
    # compute the full (qhi+qlo)@(ckhi+cklo) product with one shared weight
    # load (K=128 also enables fast-weight-load).
    qhl = nc.declare_dram_parameter("qhl", [CH, 2 * D, H * 128], bf16, isOutput=False)
    ckhh = nc.declare_dram_parameter("ckhh", [2 * D, H * M], bf16, isOutput=False)
    ckll = nc.declare_dram_parameter("ckll", [2 * D, H * M], bf16, isOutput=False)
    probs = nc.declare_dram_parameter("probs", [CH, 128, H * M], f32, isOutput=True)
    idx = nc.declare_dram_parameter("idx", [NL, TOPN], mybir.dt.uint32, isOutput=True)

    with tile.TileContext(nc) as tc:
        with (
            tc.tile_pool(name="ckpool", bufs=1) as ckpool,
            tc.tile_pool(name="qpool", bufs=3) as qpool,
            tc.tile_pool(name="epool", bufs=3) as epool,
            tc.tile_pool(name="tpool", bufs=2) as tpool,
            tc.tile_pool(name="spool", bufs=3) as spool,
            tc.tile_pool(name="pspool", bufs=8, space="PSUM") as pspool,
        ):
            ckh_sb = ckpool.tile([2 * D, H * M], bf16)
            nc.sync.dma_start(ckh_sb[:], ckhh[:])
            ckl_sb = ckpool.tile([2 * D, H * M], bf16)
            nc.sync.dma_start(ckl_sb[:], ckll[:])
            for c in range(CH):
                q_sb = qpool.tile([2 * D, H * 128], bf16)
                nc.sync.dma_start(q_sb[:], qhl[c])
                eb = epool.tile([128, H * M], f32)
                sums = spool.tile([128, H], f32)
                for h in range(H):
                    q_ap = q_sb[:, h * 128 : (h + 1) * 128]
                    ps = pspool.tile([128, M], f32)
                    nc.tensor.matmul(
                        ps[:], q_ap, ckh_sb[:, h * M : (h + 1) * M],
                        start=True, stop=False,
                    )
                    nc.tensor.matmul(
                        ps[:], q_ap, ckl_sb[:, h * M : (h + 1) * M],
                        start=False, stop=True,
                    )
                    nc.scalar.activation(
                        eb[:, h * M : (h + 1) * M],
                        ps[:],
                        mybir.ActivationFunctionType.Exp,
                        accum_out=sums[:, h : h + 1],
                    )
                rec = spool.tile([128, H], f32)
                nc.vector.reciprocal(rec[:], sums[:])
                for h in range(H):
                    src = eb[:, h * M : (h + 1) * M]
                    if h < 12:
                        nc.vector.tensor_scalar_mul(src, src, rec[:, h : h + 1])
                    else:
                        nc.scalar.activation(
                            src,
                            src,
                            mybir.ActivationFunctionType.Copy,
                            scale=rec[:, h : h + 1],
                        )
                nc.sync.dma_start(probs[c], eb[:])
                # head-sum reduction tree (fp32 exact)
                s1 = tpool.tile([128, H * M // 2], f32)
                nc.vector.tensor_add(s1[:], eb[:, : H * M // 2], eb[:, H * M // 2 :])
                nc.vector.tensor_add(s1[:, :1024], s1[:, :1024], s1[:, 1024:2048])
                nc.vector.tensor_add(s1[:, :512], s1[:, :512], s1[:, 512:1024])
                imp = spool.tile([128, M], f32)
                nc.vector.tensor_add(imp[:], s1[:, :256], s1[:, 256:512])
                # top-16 of 256 per row: two rounds of max8 + index + replace
                mx1 = spool.tile([128, 8], f32)
                nc.vector.max(mx1[:], imp[:])
                idxs = spool.tile([128, TOPN], mybir.dt.uint32)
                nc.vector.max_index(idxs[:, 0:8], mx1[:], imp[:])
                imp2 = spool.tile([128, M], f32)
                nc.vector.match_replace(imp2[:], mx1[:], imp[:], imm_value=-1.0)
                mx2 = spool.tile([128, 8], f32)
                nc.vector.max(mx2[:], imp2[:])
                nc.vector.max_index(idxs[:, 8:16], mx2[:], imp2[:])
                nc.sync.dma_start(idx[c * 128 : (c + 1) * 128, :], idxs[:])
    nc.finalize()
    return nc


def _get_nc():
    if "nc" not in _NC_CACHE:
        _NC_CACHE["nc"] = _build()
    return _NC_CACHE["nc"]


def _bf16_split(x):
    import ml_dtypes

    hi = x.astype(ml_dtypes.bfloat16)
    lo = (x - hi.astype(np.float32)).astype(ml_dtypes.bfloat16)
    return hi, lo


def _make_in_maps(q, compressed_k):
    in_maps = []
    for core in range(NCORES):
        b, half = core // 2, core % 2
        qs = q[b, :, half * NL : (half + 1) * NL, :] * np.float32(SCALE)  # (H,NL,D)
        qc = qs.reshape(H, CH, 128, D).transpose(1, 3, 0, 2).reshape(CH, D, H * 128)
        ckT = compressed_k[b].transpose(2, 0, 1).reshape(D, H * M)
        qh, ql = _bf16_split(np.ascontiguousarray(qc))
        ckh, ckl = _bf16_split(np.ascontiguousarray(ckT))
        in_maps.append(
            {
                "qhl": np.ascontiguousarray(
                    np.concatenate([qh, ql], axis=1)
                ),  # (CH, 2D, H*128)
                "ckhh": np.ascontiguousarray(np.concatenate([ckh, ckh], axis=0)),
                "ckll": np.ascontiguousarray(np.concatenate([ckl, ckl], axis=0)),
            }
        )
    return in_maps


def _run(q, compressed_k, trace=False):
    from concourse.bass_utils import run_bass_kernel_spmd

    nc = _get_nc()
    in_maps = _make_in_maps(q, compressed_k)
    res = run_bass_kernel_spmd(
        nc, in_maps, core_ids=list(range(NCORES)), trace=trace
    )
    cs = np.empty((B, H, N, M), np.float32)
    ti = np.empty((B, N, TOPN), np.int32)
    for core in range(NCORES):
        b, half = core // 2, core % 2
        pr = res.results[core]["probs"]  # (CH, 128, H*M)
        pr = pr.reshape(CH, 128, H, M).transpose(2, 0, 1, 3).reshape(H, NL, M)
        cs[b, :, half * NL : (half + 1) * NL, :] = pr
        ti[b, half * NL : (half + 1) * NL, :] = res.results[core]["idx"].astype(
            np.int32
        )
    return (ti, cs), res


def kernel(q, compressed_k, k=None, v=None):
    q = np.ascontiguousarray(np.asarray(q, dtype=np.float32))
    compressed_k = np.ascontiguousarray(np.asarray(compressed_k, dtype=np.float32))
    (ti, cs), _ = _run(q, compressed_k, trace=False)
    return ti, cs
